# revision 50
# baseline (speedup 1.0000x reference)
"""MLP (additive / Bahdanau-style) attention on 8 TRN2 NeuronCores.

Reference computation (per batch b):
    pq[h]     = sum_q query[b,q] * W_query[q,h]                  # [H]
    energy    = tanh(pk[b,s,h] + pq[h])                          # [S,H]
    scores[s] = sum_h energy[s,h] * W_energy[h]                  # [S]
    scores    = softmax(where(mask, scores, -inf))               # [S]
    context   = sum_s scores[s] * values[b,s,v]                  # [V]

Sharding: data-parallel over batch. B=32 -> 4 batches per core on 8 cores.
No collectives needed.

Per-core device kernel layout choices:
  * keys/values tiles are [128(s), 4, 1024(h)] -- natural DRAM layout,
    contiguous 4KB runs per partition, 2MB per DMA.
  * pq broadcast across partitions via a K=1 matmul with a ones row.
  * add(+pq) on DVE (in-place), tanh on ACT (in-place), then the weighted
    h-reduction as ONE fused DVE op (tensor_tensor_reduce: mult + row-sum)
    -> scores land as [128, 16] columns (s = t*128 + p).
  * softmax without max-subtraction (scores are O(+-5); exp is safe in fp32,
    matches jax softmax to rounding error). Cross-partition normalizer via a
    ones-column matmul.
  * context = probs @ values as 32 PE matmuls (lhsT = probs column [128,1],
    rhs = values tile [128,512] viewed as float32r -> full PE rate).
"""

import numpy as np

try:
    import concourse.bass as bass
except ImportError:  # pragma: no cover - container path
    import sys

    sys.path.insert(0, "/opt/trn_rl_repo")
    import concourse.bass as bass

from contextlib import ExitStack

import concourse.mybir as mybir
import concourse.tile as tile
from concourse import bacc
from concourse.bass_utils import run_bass_kernel_spmd
from concourse.masks import make_identity

B, S, Q, H, V = 32, 2048, 1024, 1024, 1024
NCORES = 8
BB = B // NCORES  # batches per core
P = 128  # SBUF partitions
NT = S // P  # 16 seq tiles per batch
GN = 4  # seq tiles per DMA group
NG = NT // GN  # 4 groups per batch

F32 = mybir.dt.float32
F32R = mybir.dt.float32r
AX = mybir.AxisListType
OP = mybir.AluOpType
ACTF = mybir.ActivationFunctionType

_CACHE: dict = {}


def _build(reps: int = 1) -> bass.Bass:
    """Build the per-core program. reps>1 repeats the whole computation
    (used only for slope-based HW timing; outputs are identical)."""
    nc = bacc.Bacc()

    q = nc.dram_tensor("q", [BB, Q], F32, kind="ExternalInput")
    pk = nc.dram_tensor("pk", [BB, S, H], F32, kind="ExternalInput")
    vals = nc.dram_tensor("vals", [BB, S, V], F32R, kind="ExternalInput")
    madd = nc.dram_tensor("madd", [BB, P, NT], F32, kind="ExternalInput")
    ebsel = nc.dram_tensor("ebsel", [BB, BB, P], F32R, kind="ExternalInput")
    wq = nc.dram_tensor("wq", [Q, H], F32R, kind="ExternalInput")
    we = nc.dram_tensor("we", [H], F32R, kind="ExternalInput")
    ctx_out = nc.dram_tensor("ctx", [BB, V], F32, kind="ExternalOutput")
    probs_out = nc.dram_tensor("probs", [BB, S], F32, kind="ExternalOutput")

    with tile.TileContext(nc) as tc, ExitStack() as ctx:
        singles = ctx.enter_context(tc.tile_pool(name="singles", bufs=1))
        pqpool = ctx.enter_context(tc.tile_pool(name="pqpool", bufs=1))
        small = ctx.enter_context(tc.tile_pool(name="small", bufs=3))
        outp = ctx.enter_context(tc.tile_pool(name="outp", bufs=2))
        ps_ctx = ctx.enter_context(tc.tile_pool(name="ps_ctx", bufs=2, space="PSUM"))
        ps_misc = ctx.enter_context(tc.tile_pool(name="ps_misc", bufs=2, space="PSUM"))
        wqp = ctx.enter_context(tc.tile_pool(name="wqp", bufs=2))
        keys = ctx.enter_context(tc.tile_pool(name="keys", bufs=4))
        vblk = ctx.enter_context(tc.tile_pool(name="vblk", bufs=4))
        ps_a = ctx.enter_context(tc.tile_pool(name="ps_a", bufs=1, space="PSUM"))

        # ---- constants ----------------------------------------------------
        ident = singles.tile([P, P], F32)
        make_identity(nc, ident)
        ones_row32 = singles.tile([1, P], F32)
        nc.vector.memset(ones_row32, 1.0)
        ones_row = singles.tile([1, P], F32R)  # K=1 lhsT: broadcast over parts
        nc.vector.tensor_copy(ones_row, ones_row32)
        ones_col = singles.tile([P, 1], F32)  # lhsT: sum over partitions
        nc.vector.memset(ones_col, 1.0)
        # one-hot selector rows: eb_all[:, b, :] is [BB, P] with row b all-ones;
        # used as lhsT to broadcast row b of a [BB, H] tensor to 128 partitions
        eb_all = singles.tile([BB, BB, P], F32R)
        nc.sync.dma_start(eb_all, ebsel[:, :, :])

        # W_energy broadcast to all partitions: [128, H]
        we_row = singles.tile([1, H], F32R)
        nc.sync.dma_start(we_row, we[None, :])
        we_bc = singles.tile([P, H], F32)
        ps = ps_a.tile([P, H], F32, tag="a")
        for j in range(2):
            sl = slice(j * 512, (j + 1) * 512)
            nc.tensor.matmul(
                ps[:, sl],
                ones_row,
                we_row[:, sl],
                start=True,
                stop=True,
            )
        nc.scalar.copy(we_bc, ps)

        # ---- pq = q @ wq for all local batches: [BB, H] --------------------
        qsb = singles.tile([BB, Q], F32)
        nc.sync.dma_start(qsb, q[:, :])
        qT = singles.tile([P, Q // P, BB], F32R)
        for c in range(Q // P):
            pst = ps_misc.tile([P, BB], F32, tag="m")
            nc.tensor.transpose(pst, qsb[:, c * P : (c + 1) * P], ident[:BB, :BB])
            nc.vector.tensor_copy(qT[:, c, :], pst)
        # Wq in 2 big DMAs so pool-slot recycling never throttles the
        # startup-critical weight stream
        pq_ps = ps_a.tile([BB, H], F32, tag="a")
        wq_v = wq.rearrange("(a c p) h -> a p c h", p=P, c=4)
        wq_last_dma = None
        for a in range(2):
            wqt = wqp.tile([P, 4, H], F32R, tag="wq")
            wq_last_dma = nc.sync.dma_start(wqt, wq_v[a])
            for ci in range(4):
                c = a * 4 + ci
                for j in range(2):
                    sl = slice(j * 512, (j + 1) * 512)
                    nc.tensor.matmul(
                        pq_ps[:, sl],
                        qT[:, c, :],
                        wqt[:, ci, sl],
                        start=(c == 0),
                        stop=(c == Q // P - 1),
                    )
        pq_sb = singles.tile([BB, H], F32R)
        nc.scalar.copy(pq_sb, pq_ps)

        # ---- broadcast pq[b] across all 128 partitions, for every batch ---
        # (hoisted before the batch loop so batch b+1's scores pass never
        # waits on batch b's context matmuls through the PE program order)
        pqbc_all = []
        pq_done = None
        for b in range(BB):
            pqbc_ps = ps_a.tile([P, H], F32, tag="a")
            for j in range(2):
                sl = slice(j * 512, (j + 1) * 512)
                nc.tensor.matmul(
                    pqbc_ps[:, sl],
                    eb_all[:, b, :],
                    pq_sb[:, sl],
                    start=True,
                    stop=True,
                )
            pqbc_b = pqpool.tile([P, H], F32, tag=f"pqbc{b}")
            pq_done = nc.scalar.copy(pqbc_b, pqbc_ps)
            pqbc_all.append(pqbc_b)

        # ---- main per-batch loop ------------------------------------------
        # Each batch's normalizer + outputs ("epilogue") are emitted in the
        # MIDDLE of the next batch's scores loop: the epilogue's ctx-scale
        # waits on the batch's last context matmul, and emitting it directly
        # before tanh(b+1) would head-of-line block the ACT queue.
        pending_epi = None
        for bi, b in enumerate([b for _ in range(reps) for b in range(BB)]):
            pqbc = pqbc_all[b]
            madd_sb = small.tile([P, NT], F32, tag="madd")
            nc.sync.dma_start(madd_sb, madd[b])
            scols = small.tile([P, NT], F32, tag="scols")
            ecols = small.tile([P, NT], F32R, tag="ecols")
            ctx_ps = ps_ctx.tile([1, V], F32, tag="ctx")

            pk_b = pk[b].rearrange("(g n p) h -> g p n h", p=P, n=GN)
            va_b = vals[b].rearrange("(g n p) h -> g p n h", p=P, n=GN)
            for g in range(NG):
                kt = keys.tile([P, GN, H], F32, tag="keys")
                nc.sync.dma_start(kt, pk_b[g])
                vt = vblk.tile([P, GN, V], F32R, tag="vals")
                vdma = nc.gpsimd.dma_start(vt, va_b[g])
                if bi == 0:
                    # keep the eager values prefetch from starving the
                    # startup-critical Wq weight stream
                    tile.add_dep_helper(
                        vdma.ins, wq_last_dma.ins, reason="values wait for wq"
                    )
                nc.vector.tensor_tensor(
                    kt[:], kt[:], pqbc[:, None, :].to_broadcast((P, GN, H)), OP.add
                )
                nc.scalar.activation(kt[:], kt[:], ACTF.Tanh)
                for n in range(GN):
                    t = g * GN + n
                    # out = (energy * 1.0) * we_bc; accum_out = row-sum(out)
                    nc.vector.scalar_tensor_tensor(
                        out=kt[:, n, :],
                        in0=kt[:, n, :],
                        scalar=1.0,
                        in1=we_bc[:],
                        op0=OP.mult,
                        op1=OP.mult,
                        accum_out=scols[:, t : t + 1],
                    )
                    # exp of this score column, mask folded in as ACT bias;
                    # then this seq tile's UNNORMALIZED context contribution
                    # immediately (float32r matmuls at full PE rate) -- no
                    # end-of-batch values burst, no output tail
                    nc.scalar.activation(
                        ecols[:, t : t + 1],
                        scols[:, t : t + 1],
                        ACTF.Exp,
                        bias=madd_sb[:, t : t + 1],
                    )
                    lhs = ecols[:, t : t + 1]
                    for j in range(2):
                        sl = slice(j * 512, (j + 1) * 512)
                        nc.tensor.matmul(
                            ctx_ps[:, sl],
                            lhs,
                            vt[:, n, sl],
                            start=(t == 0),
                            stop=(t == NT - 1),
                        )
                if g == 1 and pending_epi is not None:
                    pending_epi()
                    pending_epi = None

            def make_epilogue(b=b, ctx_ps=ctx_ps, ecols=ecols):
                def epilogue():
                    # normalizer: Z = sum of all masked exp scores
                    zrow = small.tile([P, 1], F32, tag="zrow")
                    etmp = small.tile([P, NT], F32, tag="etmp")
                    nc.scalar.activation(etmp, ecols, ACTF.Copy, accum_out=zrow)
                    zps = ps_misc.tile([1, 1], F32, tag="m")
                    nc.tensor.matmul(zps, ones_col, zrow, start=True, stop=True)
                    invz = small.tile([1, 1], F32, tag="invz")
                    nc.vector.reciprocal(invz, zps)

                    # context output: scale the raw context by 1/Z
                    ctx_sb = outp.tile([1, V], F32, tag="ctx")
                    nc.scalar.activation(ctx_sb, ctx_ps, ACTF.Copy, scale=invz[:])
                    # output DMAs on the gpsimd queue: on the sync queue they
                    # would head-of-line block later keys DMAs
                    nc.gpsimd.dma_start(ctx_out[b][None, :], ctx_sb)

                    # normalized probs output, transposed to [16, 128] so the
                    # DRAM write is contiguous
                    invbc_ps = ps_misc.tile([P, 1], F32, tag="m")
                    nc.tensor.matmul(
                        invbc_ps, ones_row32, invz, start=True, stop=True
                    )
                    invbc = small.tile([P, 1], F32, tag="invbc")
                    nc.scalar.copy(invbc, invbc_ps)
                    pcols = small.tile([P, NT], F32, tag="pcols")
                    nc.vector.tensor_scalar_mul(pcols, ecols, invbc)
                    prT_ps = ps_misc.tile([NT, P], F32, tag="m")
                    nc.tensor.transpose(prT_ps, pcols, ident)
                    prT = outp.tile([NT, P], F32, tag="prT")
                    nc.scalar.copy(prT, prT_ps)
                    nc.gpsimd.dma_start(
                        probs_out[b].rearrange("(t p) -> t p", p=P), prT
                    )

                return epilogue

            pending_epi = make_epilogue()
        pending_epi()

    nc.compile()
    nc.finalize()
    return nc


def _make_ebsel() -> np.ndarray:
    return np.ascontiguousarray(
        np.broadcast_to(np.eye(BB, dtype=np.float32)[:, :, None], (BB, BB, P))
    )


def _get_nc(reps: int = 1) -> bass.Bass:
    key = f"nc{reps}"
    if key not in _CACHE:
        _CACHE[key] = _build(reps)
    return _CACHE[key]


def kernel(query, projected_keys, values, mask, W_query, W_energy, **run_kwargs):
    q = np.ascontiguousarray(np.asarray(query, dtype=np.float32).reshape(B, Q))
    pk = np.ascontiguousarray(np.asarray(projected_keys, dtype=np.float32))
    va = np.ascontiguousarray(np.asarray(values, dtype=np.float32))
    m = np.asarray(mask).astype(bool)
    madd = np.where(m, np.float32(0.0), np.float32(-1e30)).astype(np.float32)
    madd = np.ascontiguousarray(madd.reshape(B, NT, P).transpose(0, 2, 1))
    wq = np.ascontiguousarray(np.asarray(W_query, dtype=np.float32))
    we = np.ascontiguousarray(np.asarray(W_energy, dtype=np.float32).reshape(H))
    ebsel = _make_ebsel()

    nc = _get_nc()
    in_maps = []
    for c in range(NCORES):
        sl = slice(c * BB, (c + 1) * BB)
        in_maps.append(
            dict(
                q=q[sl], pk=pk[sl], vals=va[sl], madd=madd[sl],
                ebsel=ebsel, wq=wq, we=we,
            )
        )
    res = run_bass_kernel_spmd(nc, in_maps, core_ids=list(range(NCORES)), **run_kwargs)
    if run_kwargs:
        _CACHE["last_result"] = res
    context = np.concatenate([r["ctx"] for r in res.results], axis=0)
    probs = np.concatenate([r["probs"] for r in res.results], axis=0)
    return context.reshape(B, V), probs.reshape(B, S)
